# revision 1
# baseline (speedup 1.0000x reference)
"""Trainium2 Bass kernel for nn_DensityEdgeProjection.

Strategy (sharding_hint): the ns*nb*nb density-element (KV token) axis is
sharded over 8 cores as 768 (spin, i) rows of 384 j-tokens each -> 96 rows
per core.  Each core encodes its token slice and produces partial attention
numerators/denominators (no max -- scores are provably tiny), which the host
combines (flash-attention style) and pushes through the small output MLP.

Device layout: activations are kept feature-major (features on partitions,
tokens on the free axis) so the whole encoder + K/V/S chain needs zero
transposes.  All matmuls are bf16 with fp32 PSUM accumulation.
"""

import os
import sys
import numpy as np

sys.path.insert(0, "/opt/trn_rl_repo")

from ml_dtypes import bfloat16

NB = 384
NS = 2
D = 256
TQ = 32
H = 8
DH = 32
MAX_L = 2
NCORES = 8
NROW_TOTAL = NS * NB          # 768 (s, i) rows
NROW = NROW_TOTAL // NCORES   # 96 rows per core
EPS = 1e-5

LAST_EXEC_NS = None
LAST_RESULTS = None

_PROGRAM = None


def _np_silu(x):
    return x / (1.0 + np.exp(-x))


def _np_layernorm(x, w, b):
    mu = x.mean(-1, keepdims=True)
    var = x.var(-1, keepdims=True)
    return (x - mu) / np.sqrt(var + EPS) * w + b


def _blob_layout(nrow):
    """element offsets (bf16) for each packed section, 512-aligned."""
    sections = [
        ("rho", (nrow, 3, NB)),
        ("lA", (nrow, 3, D)),
        ("ajT", (2, 128, NB)),
        ("w2", (128, 512)),
        ("w3", (128, 512)),
        ("wq", (128, 512)),
        ("wv", (128, 512)),
        ("dneg", (1, 256)),
        ("svneg", (1, 256)),
        ("onesw", (128, 1)),
        ("ident", (128, 128)),
    ]
    offs = {}
    off = 0
    for k, shp in sections:
        n = int(np.prod(shp))
        offs[k] = (off, n, shp)
        off += (n + 511) // 512 * 512
    return offs, off


def _pack_blob(parts, nrow):
    offs, total = _blob_layout(nrow)
    blob = np.zeros(total, bfloat16)
    for k, arr in parts.items():
        off, n, shp = offs[k]
        assert tuple(arr.shape) == tuple(shp), (k, arr.shape, shp)
        blob[off:off + n] = arr.astype(bfloat16).ravel()
    return blob


def _build_program(nrow=NROW, debug_taps=False, repeat=1):
    """Build the (single, shared across cores) Bass/Tile program.

    Software-pipelined: row r's encoder matmuls interleave with row r-1's
    attention matmuls so the PE never stalls on same-row elementwise chains.
    """
    NROW_L = nrow
    import concourse.bass as bass
    import concourse.bacc as bacc
    import concourse.tile as tile
    from concourse import mybir

    f32 = mybir.dt.float32
    bf16 = mybir.dt.bfloat16
    AF = mybir.ActivationFunctionType
    OP = mybir.AluOpType

    nc = bacc.Bacc("TRN2", target_bir_lowering=False, debug=False,
                   num_devices=NCORES)

    # ---- kernel I/O: one packed bf16 blob in, one f32 tensor out ----
    offs, total = _blob_layout(NROW_L)
    blob_d = nc.dram_tensor("blob", [total], bf16, kind="ExternalInput")
    out_d = nc.dram_tensor("opart", [128, 514], f32, kind="ExternalOutput")

    def bslice(key, idx=None):
        off, n, shape = offs[key]
        if idx is not None:
            per = shape[-2] * shape[-1]
            off = off + idx * per
            n = per
            shape = shape[-2:]
        ap = blob_d[off:off + n]
        return ap.rearrange("(p n) -> p n", p=shape[0])

    if debug_taps:
        dbg = {
            "dbg_x": nc.dram_tensor("dbg_x", [128, 896], bf16, kind="ExternalOutput"),
            "dbg_srow": nc.dram_tensor("dbg_srow", [1, 896], bf16, kind="ExternalOutput"),
            "dbg_rstd": nc.dram_tensor("dbg_rstd", [128, 3], f32, kind="ExternalOutput"),
            "dbg_pT": nc.dram_tensor("dbg_pT", [128, 256], bf16, kind="ExternalOutput"),
            "dbg_vsb": nc.dram_tensor("dbg_vsb", [128, 260], bf16, kind="ExternalOutput"),
        }

    with tile.TileContext(nc) as tc:
        with (
            tc.tile_pool(name="const", bufs=1) as cpool,
            tc.tile_pool(name="io", bufs=8) as iopool,
            tc.tile_pool(name="work", bufs=4) as wpool,
            tc.tile_pool(name="attw", bufs=4) as apool,
            tc.tile_pool(name="mlppsum", bufs=int(os.environ.get("PS_MLP", "3")), space="PSUM") as mlpp,
            tc.tile_pool(name="spsum", bufs=int(os.environ.get("PS_S", "1")), space="PSUM") as spool,
            tc.tile_pool(name="smallpsum", bufs=int(os.environ.get("PS_SM", "2")), space="PSUM") as vpool,
            tc.tile_pool(name="opsum", bufs=1, space="PSUM") as opool,
        ):
            # ---- load constants ----
            ajT0 = cpool.tile([128, NB], bf16)
            ajT1 = cpool.tile([128, NB], bf16)
            nc.sync.dma_start(ajT0[:], bslice("ajT", 0))
            nc.sync.dma_start(ajT1[:], bslice("ajT", 1))
            ajT = [ajT0, ajT1]
            w2_s = cpool.tile([128, 512], bf16)
            w3_s = cpool.tile([128, 512], bf16)
            wq_s = cpool.tile([128, 512], bf16)
            wv_s = cpool.tile([128, 512], bf16)
            for t, k in ((w2_s, "w2"), (w3_s, "w3"), (wq_s, "wq"), (wv_s, "wv")):
                nc.sync.dma_start(t[:], bslice(k))
            dneg_s = cpool.tile([1, 256], bf16)
            svneg_s = cpool.tile([1, 256], bf16)
            ones_s = cpool.tile([128, 1], bf16)
            id_s = cpool.tile([128, 128], bf16)
            id32_s = cpool.tile([1, 1], f32)
            nc.vector.memset(id32_s[:], 1.0)
            nc.sync.dma_start(dneg_s[:], bslice("dneg"))
            nc.sync.dma_start(svneg_s[:], bslice("svneg"))
            nc.sync.dma_start(ones_s[:], bslice("onesw"))
            nc.sync.dma_start(id_s[:], bslice("ident"))

            # ---- persistent attention accumulators ----
            o_ps0 = opool.tile([128, 257], f32)
            o_ps1 = opool.tile([128, 257], f32)
            o_ps = [o_ps0, o_ps1]

            NIT = repeat * NROW_L

            def emit_Smm(ctx, rs):
                """scores for one 128-token sub-block of a finished row."""
                u = ctx["uid"]
                x_sb = ctx["x"]
                s_ps = spool.tile([128, 256], f32, tag="s", name=f"s{u}_{rs}")
                for fc in range(2):
                    nc.tensor.matmul(s_ps[:],
                                     x_sb[:, fc * 512 + rs * 128:
                                          fc * 512 + rs * 128 + 128],
                                     wq_s[:, fc * 256:(fc + 1) * 256],
                                     start=(fc == 0), stop=(fc == 1))
                ctx["s_ps"][rs] = s_ps

            def emit_exp(ctx, rs):
                u = ctx["uid"]
                pT = apool.tile([128, 256], bf16, tag="pT", name=f"p{u}_{rs}")
                nc.scalar.activation(pT[:], ctx["s_ps"][rs][:], AF.Exp,
                                     scale=ctx["rstd"][:, rs:rs + 1])
                ctx["pT"][rs] = pT
                if debug_taps and ctx["first"] and rs == 0:
                    nc.sync.dma_start(dbg["dbg_pT"][:], pT[:])

            def emit_V(ctx, rs):
                u = ctx["uid"]
                x_sb = ctx["x"]
                v_ps = vpool.tile([128, 256], f32, tag="sm", name=f"v{u}_{rs}")
                for fc in range(2):
                    nc.tensor.matmul(v_ps[:],
                                     x_sb[:, fc * 512 + rs * 128:
                                          fc * 512 + rs * 128 + 128],
                                     wv_s[:, fc * 256:(fc + 1) * 256],
                                     start=(fc == 0), stop=(fc == 1))
                v_sb = apool.tile([128, 260], bf16, tag="vsb", name=f"v{u}_{rs}")
                nc.vector.tensor_scalar(v_sb[:, 0:256], v_ps[:],
                                        ctx["rstd"][:, rs:rs + 1], None,
                                        OP.mult)
                nc.gpsimd.memset(v_sb[:, 256:257], 1.0)
                ctx["v"][rs] = v_sb
                if debug_taps and ctx["first"] and rs == 0:
                    nc.sync.dma_start(dbg["dbg_vsb"][:], v_sb[:])

            def emit_O(ctx, rs):
                pT = ctx["pT"][rs]
                v_sb = ctx["v"][rs]
                for oc in range(2):
                    nc.tensor.matmul(o_ps[oc][:, 0:257],
                                     pT[:, oc * 128:(oc + 1) * 128],
                                     v_sb[:, 0:257],
                                     start=ctx["first"] and rs == 0,
                                     stop=ctx["last"] and rs == 2)

            prev = None
            for it in range(NIT + 1):
                cur = None
                if it < NIT:
                    rep, r = divmod(it, NROW_L)
                    uid = f"{rep}_{r}"
                    cur = {"uid": uid, "first": it == 0, "last": it == NIT - 1,
                           "pT": [None] * 3, "v": [None] * 3, "s_ps": [None] * 3}

                    rho_t = iopool.tile([3, NB], bf16, tag="rho", name=f"rh{uid}")
                    nc.sync.dma_start(rho_t[:], bslice("rho", r))
                    lA_t = iopool.tile([3, D], bf16, tag="lA", name=f"lA{uid}")
                    nc.sync.dma_start(lA_t[:], bslice("lA", r))

                    # encoder layer 1 preactivation (feature-major)
                    hpre = [mlpp.tile([128, NB], f32, tag="mlp",
                                      name=f"hp{uid}_{c}") for c in range(2)]
                    for c in range(2):
                        nc.tensor.matmul(hpre[c][:],
                                         lA_t[:, c * 128:(c + 1) * 128],
                                         rho_t[:], start=True, stop=False)
                        nc.tensor.matmul(hpre[c][:], id_s[:], ajT[c][:],
                                         start=False, stop=True)
                    if prev is not None:
                        emit_Smm(prev, 0)
                    t1 = wpool.tile([128, 896], bf16, tag="tanh", name=f"t1{uid}")
                    hhat = wpool.tile([128, 896], bf16, tag="act", name=f"hh{uid}")
                    for c in range(2):
                        sl = slice(c * 512, c * 512 + NB)
                        nc.scalar.activation(t1[:, sl], hpre[c][:], AF.Tanh,
                                             scale=0.5)
                        nc.vector.scalar_tensor_tensor(hhat[:, sl], t1[:, sl],
                                                       1.0, hpre[c][:],
                                                       OP.add, OP.mult)
                    if prev is not None:
                        emit_exp(prev, 0)

                    # encoder layer 2
                    l2p = [mlpp.tile([128, NB], f32, tag="mlp",
                                     name=f"l2{uid}_{c}") for c in range(2)]
                    for oc in range(2):
                        for fc in range(2):
                            nc.tensor.matmul(
                                l2p[oc][:],
                                w2_s[:, fc * 256 + oc * 128:
                                     fc * 256 + (oc + 1) * 128],
                                hhat[:, fc * 512: fc * 512 + NB],
                                start=(fc == 0), stop=(fc == 1))
                    if prev is not None:
                        emit_Smm(prev, 1)
                        emit_V(prev, 0)
                    t2 = wpool.tile([128, 896], bf16, tag="tanh", name=f"t2{uid}")
                    that = wpool.tile([128, 896], bf16, tag="act", name=f"th{uid}")
                    for c in range(2):
                        sl = slice(c * 512, c * 512 + NB)
                        nc.scalar.activation(t2[:, sl], l2p[c][:], AF.Tanh,
                                             scale=0.5)
                        nc.vector.scalar_tensor_tensor(that[:, sl], t2[:, sl],
                                                       1.0, l2p[c][:],
                                                       OP.add, OP.mult)
                    if prev is not None:
                        emit_exp(prev, 1)

                    # encoder layer 3 -> x = kv_pre
                    xps = [mlpp.tile([128, NB], f32, tag="mlp",
                                     name=f"xp{uid}_{c}") for c in range(2)]
                    for oc in range(2):
                        for fc in range(2):
                            nc.tensor.matmul(
                                xps[oc][:],
                                w3_s[:, fc * 256 + oc * 128:
                                     fc * 256 + (oc + 1) * 128],
                                that[:, fc * 512: fc * 512 + NB],
                                start=(fc == 0), stop=(fc == 1))
                    if prev is not None:
                        emit_Smm(prev, 2)
                        emit_V(prev, 1)
                        emit_V(prev, 2)
                    x_sb = wpool.tile([128, 896], bf16, tag="x", name=f"x{uid}")
                    x2_sb = wpool.tile([128, 896], bf16, tag="x2", name=f"x2{uid}")
                    for c in range(2):
                        sl = slice(c * 512, c * 512 + NB)
                        nc.scalar.activation(x_sb[:, sl], xps[c][:], AF.Identity)
                        nc.gpsimd.tensor_mul(x2_sb[:, sl], x_sb[:, sl],
                                              x_sb[:, sl])
                    cur["x"] = x_sb
                    if prev is not None:
                        emit_exp(prev, 2)
                        emit_O(prev, 0)
                        emit_O(prev, 1)
                        emit_O(prev, 2)

                    # E[x^2] columns: x2-as-lhsT ones-matmuls
                    e2col = vpool.tile([128, 4], f32, tag="sm", name=f"ec{uid}")
                    for rs in range(3):
                        for c in range(2):
                            nc.tensor.matmul(
                                e2col[:, rs:rs + 1],
                                x2_sb[:, c * 512 + rs * 128:
                                      c * 512 + rs * 128 + 128],
                                ones_s[:], start=(c == 0), stop=(c == 1))

                    # rstd = 1/sqrt(E[x^2] - mu^2 + eps): quake + 2 Newton
                    scr = wpool.tile([128, 20], f32, tag="scr", name=f"q{uid}")
                    v = scr[:, 3:6]
                    y = scr[:, 6:9]
                    ta = scr[:, 9:12]
                    tb = scr[:, 12:15]
                    nc.vector.tensor_scalar(v, e2col[:, 0:3], EPS, None,
                                            OP.add)
                    v_u = v.bitcast(mybir.dt.uint32)
                    y_u = y.bitcast(mybir.dt.uint32)
                    nc.vector.tensor_scalar(y_u, v_u, 1, None,
                                            OP.logical_shift_right)
                    nc.vector.tensor_scalar(y_u, y_u, 0xA0C8A620, None, OP.add)
                    nc.vector.tensor_scalar(y_u, y_u, 0xFFFFFFFF, None,
                                            OP.bitwise_xor)
                    rstd = wpool.tile([128, 3], f32, tag="rstd", name=f"rs{uid}")
                    for nit in range(2):
                        dst = y if nit == 0 else rstd[:, 0:3]
                        nc.vector.tensor_mul(ta, y, y)
                        nc.vector.tensor_mul(tb, ta, v)
                        nc.vector.tensor_scalar(ta, tb, -0.5, 1.5,
                                                OP.mult, OP.add)
                        nc.vector.tensor_mul(dst, y, ta)
                    cur["rstd"] = rstd

                    if debug_taps and it == 0:
                        nc.sync.dma_start(dbg["dbg_x"][:], x_sb[:])
                        nc.sync.dma_start(dbg["dbg_rstd"][:], rstd[:])
                else:
                    # drain: final row's attention
                    for rs in range(3):
                        emit_Smm(prev, rs)
                        emit_exp(prev, rs)
                        emit_V(prev, rs)
                    for rs in range(3):
                        emit_O(prev, rs)
                prev = cur if cur is not None else prev

            # ---- write out partial results ----
            ostage = cpool.tile([128, 514], f32)
            nc.vector.tensor_copy(ostage[:, 0:257], o_ps0[:])
            nc.vector.tensor_copy(ostage[:, 257:514], o_ps1[:])
            nc.sync.dma_start(out_d[:], ostage[:])

    nc.compile()
    return nc


def _get_program():
    global _PROGRAM
    if _PROGRAM is None:
        _PROGRAM = _build_program()
    return _PROGRAM


def build_in_maps(inp):
    f = np.float32

    # ---------------- host precompute (tiny, O(nb*D^2)) ----------------
    Z = inp["Z"].astype(np.int64)
    ang_l = inp["ang_l"].astype(np.int64)
    m_sh = np.clip(inp["mag_m"].astype(np.int64) + MAX_L, 0, 2 * MAX_L)
    orb_in = np.concatenate([inp["elem_emb"][Z], inp["l_emb"][ang_l],
                             inp["m_emb"][m_sh]], axis=-1).astype(f)
    orb = (_np_silu(orb_in @ inp["proj_w1"] + inp["proj_b1"])
           @ inp["proj_w2"] + inp["proj_b2"]).astype(f)

    enc_w1 = inp["enc_w1"].astype(f)
    a_i = orb @ enc_w1[:128]
    a_j = orb @ enc_w1[128:256]
    w_r = enc_w1[256]
    w_im = enc_w1[257]
    a_ib = a_i + inp["enc_b1"].astype(f)

    if not (np.all(inp["enc_b2"] == 0) and np.all(inp["enc_b3"] == 0)):
        raise NotImplementedError("nonzero enc_b2/enc_b3 not supported")

    lnw = inp["ln_kv_w"].astype(f)
    lnb = inp["ln_kv_b"].astype(f)
    wk_p = lnw[:, None] * inp["wk"].astype(f)
    wv_p = lnw[:, None] * inp["wv"].astype(f)

    qn = _np_layernorm(inp["query_tokens"].astype(f), inp["ln_q_w"].astype(f),
                       inp["ln_q_b"].astype(f))
    Q = (qn @ inp["wq"].astype(f) + inp["bq"].astype(f)).reshape(TQ, H, DH)

    WQ = np.zeros((D, D), f)
    for h in range(H):
        WQ[:, h * DH:(h + 1) * DH] = (wk_p[:, h * DH:(h + 1) * DH]
                                      @ Q[:, h, :].T) / np.sqrt(DH)
    d_row = WQ.sum(0)
    sv_row = wv_p.sum(0)

    w2p = 0.5 * inp["enc_w2"].astype(f)
    # centering w3's output columns makes kv_pre exactly zero-mean over
    # features, which is what layernorm subtracts -- mu pipeline vanishes.
    w3p = 0.5 * inp["enc_w3"].astype(f)
    w3p = w3p - w3p.mean(axis=1, keepdims=True)

    def pack_w(w):  # [256, 256] -> [128, 512] (f-chunks side by side)
        return np.concatenate([w[0:128, :], w[128:256, :]], axis=1)

    # per-core sharded rho rows + layer-1 lhsT
    rho_r = inp["rho_real"].astype(f).reshape(NROW_TOTAL, NB)
    rho_i = inp["rho_imag"].astype(f).reshape(NROW_TOTAL, NB)

    common = {
        "ajT": np.ascontiguousarray(a_j.T.reshape(2, 128, NB)),
        "w2": pack_w(w2p),
        "w3": pack_w(w3p),
        "wq": pack_w(WQ),
        "wv": pack_w(wv_p),
        "dneg": (-d_row).reshape(1, 256),
        "svneg": (-sv_row).reshape(1, 256),
        "onesw": np.full((128, 1), 1.0 / 256.0, f),
        "ident": np.eye(128, dtype=f),
    }

    in_maps = []
    for c in range(NCORES):
        rows = slice(c * NROW, (c + 1) * NROW)
        g = np.arange(c * NROW, (c + 1) * NROW)
        i_idx = g % NB
        rho = np.empty((NROW, 3, NB), f)
        rho[:, 0, :] = rho_r[rows]
        rho[:, 1, :] = rho_i[rows]
        rho[:, 2, :] = 1.0
        lA = np.empty((NROW, 3, D), f)
        lA[:, 0, :] = w_r
        lA[:, 1, :] = w_im
        lA[:, 2, :] = a_ib[i_idx]
        parts = dict(common)
        parts["rho"] = rho
        parts["lA"] = lA
        in_maps.append({"blob": _pack_blob(parts, NROW)})
    return in_maps


def combine_results(inp, core_results):
    f = np.float32
    num = np.zeros((H, TQ, DH), np.float64)
    den = np.zeros((H, TQ), np.float64)
    for c in range(NCORES):
        arr = np.asarray(core_results[c]["opart"], f)
        for h in range(H):
            oc, hrel = divmod(h, 4)
            blk = arr[:, oc * 257:(oc + 1) * 257]
            rows_ = slice(hrel * TQ, (hrel + 1) * TQ)
            num[h] += blk[rows_, h * DH:(h + 1) * DH]
            den[h] += blk[rows_, 256]

    lnb = inp["ln_kv_b"].astype(f)
    ctx = np.empty((TQ, D), f)
    for h in range(H):
        ctx[:, h * DH:(h + 1) * DH] = (num[h] / den[h][:, None]).astype(f)
    cv = inp["wv"].astype(f).T @ lnb + inp["bv"].astype(f)
    ctx = ctx + cv

    attended = ctx @ inp["wo"].astype(f) + inp["bo"].astype(f)
    y = (_np_silu(attended @ inp["out_w1"].astype(f) + inp["out_b1"].astype(f))
         @ inp["out_w2"].astype(f) + inp["out_b2"].astype(f))
    return y.astype(np.float32)


def kernel(**inputs):
    global LAST_EXEC_NS, LAST_RESULTS
    inp = {k: np.asarray(v) for k, v in inputs.items()}
    in_maps = build_in_maps(inp)

    # ---------------- run on the 8 NeuronCores ----------------
    from concourse.bass_utils import run_bass_kernel_spmd

    nc = _get_program()
    trace = bool(int(os.environ.get("BASS_KERNEL_TRACE", "0")))
    try:
        res = run_bass_kernel_spmd(nc, in_maps, list(range(NCORES)),
                                   trace=trace)
    except Exception:
        if not trace:
            raise
        res = run_bass_kernel_spmd(nc, in_maps, list(range(NCORES)),
                                   trace=False)
    LAST_EXEC_NS = res.exec_time_ns
    LAST_RESULTS = res
    return combine_results(inp, res.results)

